# revision 45
# baseline (speedup 1.0000x reference)
"""Trainium2 Bass kernel for nn_HadaMard: fused proj + 2xLayerNorm + outer product.

Reference computation (per batch b):
  qf = q[b].reshape(C1, N)           # [1024, 1024]  (C1 on rows, N=H*W cols)
  proj = Wp @ qf + bp                # [256, 1024]
  qn = LN_over_d(proj) * g1 + b1     # LN over the 256-channel dim
  xn = LN_over_e(x[b]) * g2 + b2     # LN over the 32-channel dim
  out[d*32+e, n] = qn[d, n] * xn[e, n]   # [8192, 1024]

Sharding: data-parallel over B=8, one batch per NeuronCore.

On-chip layout is [channel, n] everywhere (zero transposes):
  - proj = WpT.T @ q via PE (WpT host-transposed, q natural layout)
  - LN stats over the partition axis via ones-matmuls: lhsT = ones*(1/C)
    gives the mean broadcast to all 128 partitions for free.
  - outer product: stationary S4 [4,128] (S4[j,p] = 1 if p//32 == j) broadcasts
    4 qn rows -> 128 partitions in PSUM; one DVE tensor_mul against a
    replicated xn tile -> output tile [128, 1024] -> contiguous 512KB DMA.
"""

import numpy as np

_CACHE = {}

B, C1, H, W = 8, 1024, 32, 32
C2 = 32
Cp = 256
N = H * W  # 1024
CD = Cp * C2  # 8192
EPS = 1e-5


def _build_nc(trace_label=False):
    import os

    import concourse.bacc as bacc
    import concourse.bass as bass
    import concourse.mybir as mybir
    import concourse.tile as tile

    f32r_proj = os.environ.get("HM_F32R_PROJ", "0") == "1"
    f32r_stats = os.environ.get("HM_F32R_STATS", "0") == "1"
    f32r_sel = os.environ.get("HM_F32R_SEL", "0") == "1"
    simple = os.environ.get("HM_SIMPLE", "0") == "1"  # g1=1,b1=0,g2=1,b2=0,bp=0
    split = os.environ.get("HM_SPLIT", "1") == "1"  # bf16 hi/lo selection matmuls

    F32 = mybir.dt.float32
    F32R = mybir.dt.float32r
    BF16 = mybir.dt.bfloat16
    MULT = mybir.AluOpType.mult
    ADD = mybir.AluOpType.add
    SQRT = mybir.ActivationFunctionType.Sqrt

    nc = bacc.Bacc(None, target_bir_lowering=False)

    q_d = nc.dram_tensor("q", [C1, N], F32, kind="ExternalInput")
    x_d = nc.dram_tensor("x", [C2, N], F32, kind="ExternalInput")
    w_d = nc.dram_tensor("wpt", [C1, Cp], F32, kind="ExternalInput")
    bp_d = nc.dram_tensor("bpc", [128, 2], F32, kind="ExternalInput")
    g1_d = nc.dram_tensor("g1c", [128, 2], F32, kind="ExternalInput")
    b1_d = nc.dram_tensor("b1c", [128, 2], F32, kind="ExternalInput")
    g2_d = nc.dram_tensor("g2r", [128, 1], F32, kind="ExternalInput")
    b2_d = nc.dram_tensor("b2r", [128, 1], F32, kind="ExternalInput")
    rep_d = nc.dram_tensor(
        "rep", [128, 16 * 128], BF16 if split else F32, kind="ExternalInput"
    )
    sx_d = nc.dram_tensor("sx", [C2, 128], F32, kind="ExternalInput")
    out_d = nc.dram_tensor("out", [CD, N], F32, kind="ExternalOutput")

    with tile.TileContext(nc) as tc:
        with (
            tc.tile_pool(name="cst", bufs=1) as cst,
            tc.tile_pool(name="big", bufs=1) as big,
            tc.tile_pool(name="wrk", bufs=2) as wrk,
            tc.tile_pool(name="stt", bufs=1) as stt,
            tc.tile_pool(name="ost", bufs=4) as ost,
            tc.tile_pool(name="ps", bufs=4, space=bass.MemorySpace.PSUM) as ps,
        ):
            # ---- input loads ----
            q_sb = []
            for k in range(8):
                t = big.tile([128, N], F32, tag=f"q{k}")
                nc.sync.dma_start(t[:], q_d[128 * k : 128 * (k + 1), :])
                q_sb.append(t)
            w_sb = []
            for k in range(8):
                t = big.tile([128, Cp], F32, tag=f"w{k}")
                nc.sync.dma_start(t[:], w_d[128 * k : 128 * (k + 1), :])
                w_sb.append(t)
            x_sb = cst.tile([C2, N], F32, tag="xs")
            nc.sync.dma_start(x_sb[:], x_d[:])

            def cload(dram, shape, tag):
                t = cst.tile(shape, F32, tag=tag)
                nc.sync.dma_start(t[:], dram[:])
                return t

            bp_sb = cload(bp_d, [128, 2], "bp")
            g1_sb = cload(g1_d, [128, 2], "g1")
            b1_sb = cload(b1_d, [128, 2], "b1")
            g2_sb = cload(g2_d, [128, 1], "g2")
            b2_sb = cload(b2_d, [128, 1], "b2")
            cq_sb = cst.tile([128, 128], F32, tag="cq")
            nc.vector.memset(cq_sb[:], 1.0 / Cp)
            cx_sb = cst.tile([C2, 128], F32, tag="cx")
            nc.vector.memset(cx_sb[:], 1.0 / C2)
            rep_sb = cst.tile([128, 16 * 128], BF16 if split else F32, tag="rep")
            nc.sync.dma_start(rep_sb[:], rep_d[:])
            sx_sb = cload(sx_d, [C2, 128], "sx")
            eps_t = cst.tile([128, 1], F32, tag="eps")
            nc.vector.memset(eps_t[:], EPS)

            def mm_dt(ap, on):
                return ap.bitcast(F32R) if on else ap

            # ---- projection: proj[d, n] = sum_c WpT[c, d] * q[c, n]  (+bp) ----
            projb = []
            for md in range(2):
                pj = ps.tile([128, N], F32, tag="ps")
                for k in range(8):
                    lhsT = w_sb[k][:, 128 * md : 128 * (md + 1)]
                    for h in range(2):
                        nc.tensor.matmul(
                            pj[:, 512 * h : 512 * (h + 1)],
                            mm_dt(lhsT, f32r_proj),
                            mm_dt(q_sb[k][:, 512 * h : 512 * (h + 1)], f32r_proj),
                            start=(k == 0),
                            stop=(k == 7),
                        )
                pb = stt.tile([128, N], F32, tag=f"pb{md}")
                if simple:
                    nc.vector.tensor_copy(pb[:], pj[:])
                else:
                    nc.vector.tensor_scalar_add(pb[:], pj[:], bp_sb[:, md : md + 1])
                projb.append(pb)

            # squares (ScalarE, keeps DVE free)
            sq = []
            for md in range(2):
                s = wrk.tile([128, N], F32, tag=f"sq{md}")
                nc.scalar.square(s[:], projb[md][:])
                sq.append(s)

            # stats via ones-matmuls: mean & E[v^2], broadcast to 128 partitions
            smq = ps.tile([128, N], F32, tag="ps")
            for md in range(2):
                for h in range(2):
                    nc.tensor.matmul(
                        smq[:, 512 * h : 512 * (h + 1)],
                        mm_dt(cq_sb[:], f32r_stats),
                        mm_dt(projb[md][:, 512 * h : 512 * (h + 1)], f32r_stats),
                        start=(md == 0),
                        stop=(md == 1),
                    )
            sqq = ps.tile([128, N], F32, tag="ps")
            for md in range(2):
                for h in range(2):
                    nc.tensor.matmul(
                        sqq[:, 512 * h : 512 * (h + 1)],
                        mm_dt(cq_sb[:], f32r_stats),
                        mm_dt(sq[md][:, 512 * h : 512 * (h + 1)], f32r_stats),
                        start=(md == 0),
                        stop=(md == 1),
                    )

            mb = stt.tile([128, N], F32, tag="mb")
            nc.vector.tensor_copy(mb[:], smq[:])
            m2 = wrk.tile([128, N], F32, tag="t")
            nc.scalar.square(m2[:], mb[:])
            var = wrk.tile([128, N], F32, tag="t2")
            nc.vector.tensor_sub(var[:], sqq[:], m2[:])
            sd = wrk.tile([128, N], F32, tag="t")
            nc.scalar.activation(sd[:], var[:], SQRT, bias=eps_t[:])
            rstd = stt.tile([128, N], F32, tag="rstd")
            rscr = wrk.tile([128, N], F32, tag="t3")
            nc.vector.reciprocal_approx_accurate(rstd[:], sd[:], rscr[:])

            # simple mode: qn holds (projb - mean); rstd is folded into XR so the
            # per-tile multiply produces (projb-m)*rstd*xn in one op.
            qn = []
            qn_lo = []
            for md in range(2):
                qq = stt.tile([128, N], F32, tag=f"qn{md}")
                nc.vector.tensor_sub(qq[:], projb[md][:], mb[:])
                if not simple:
                    nc.vector.tensor_mul(qq[:], qq[:], rstd[:])
                    nc.vector.tensor_scalar(
                        qq[:], qq[:], g1_sb[:, md : md + 1], b1_sb[:, md : md + 1],
                        op0=MULT, op1=ADD,
                    )
                if split:
                    # bf16 hi/lo decomposition: qq = hi + lo, |lo| <~ 2^-8 |qq|
                    qh = stt.tile([128, N], BF16, tag=f"qh{md}")
                    nc.vector.tensor_copy(qh[:], qq[:])
                    ql = stt.tile([128, N], BF16, tag=f"ql{md}")
                    nc.vector.tensor_sub(ql[:], qq[:], qh[:])
                    qn.append(qh)
                    qn_lo.append(ql)
                else:
                    qn.append(qq)

            # ---- x LayerNorm (over 32 channels) + partition replication ----
            xsq = wrk.tile([C2, N], F32, tag="xq")
            nc.scalar.square(xsq[:], x_sb[:])
            smx = ps.tile([128, N], F32, tag="ps")
            for h in range(2):
                nc.tensor.matmul(
                    smx[:, 512 * h : 512 * (h + 1)], mm_dt(cx_sb[:], f32r_stats),
                    mm_dt(x_sb[:, 512 * h : 512 * (h + 1)], f32r_stats),
                    start=True, stop=True,
                )
            sqx = ps.tile([128, N], F32, tag="ps")
            for h in range(2):
                nc.tensor.matmul(
                    sqx[:, 512 * h : 512 * (h + 1)], mm_dt(cx_sb[:], f32r_stats),
                    mm_dt(xsq[:, 512 * h : 512 * (h + 1)], f32r_stats),
                    start=True, stop=True,
                )
            xb = ps.tile([128, N], F32, tag="ps")
            for h in range(2):
                nc.tensor.matmul(
                    xb[:, 512 * h : 512 * (h + 1)],
                    mm_dt(sx_sb[:], f32r_sel),
                    mm_dt(x_sb[:, 512 * h : 512 * (h + 1)], f32r_sel),
                    start=True, stop=True,
                )

            mxb = wrk.tile([128, N], F32, tag="mx")
            nc.vector.tensor_copy(mxb[:], smx[:])
            mx2 = wrk.tile([128, N], F32, tag="t")
            nc.scalar.square(mx2[:], mxb[:])
            vx = wrk.tile([128, N], F32, tag="t2")
            nc.vector.tensor_sub(vx[:], sqx[:], mx2[:])
            sdx = wrk.tile([128, N], F32, tag="t")
            nc.scalar.activation(sdx[:], vx[:], SQRT, bias=eps_t[:])
            rsx = wrk.tile([128, N], F32, tag="t3")
            rscx = wrk.tile([128, N], F32, tag="t4")
            nc.vector.reciprocal_approx_accurate(rsx[:], sdx[:], rscx[:])
            xt = wrk.tile([128, N], F32, tag="t2")
            nc.vector.tensor_sub(xt[:], xb[:], mxb[:])
            xnr = stt.tile([128, N], F32, tag="xnr")
            nc.vector.tensor_mul(xnr[:], xt[:], rsx[:])
            if simple:
                # fold q-side rstd into the shared multiplier tile
                nc.vector.tensor_mul(xnr[:], xnr[:], rstd[:])
            else:
                nc.vector.tensor_scalar(
                    xnr[:], xnr[:], g2_sb[:, 0:1], b2_sb[:, 0:1], op0=MULT, op1=ADD
                )

            # ---- outer product: 64 output tiles of [128, 1024] ----
            # tile t = (md, g, r): output rows 128t..128(t+1), qn rows
            # 128md + 32g + 4r + {0..3}. lhsT and rhs share base partition 32g
            # (tile_position constraint); rep_sb holds the selection matrices
            # replicated vertically 4x so any 32-row slice works.
            out_dma_engines = [nc.sync, nc.scalar]
            ot = None
            for md in range(2):
                for g in range(2):
                    for r in range(16):
                        qb = ps.tile([128, N], F32, tag="ps")
                        lhsT = rep_sb[64 * g : 64 * (g + 1), 128 * r : 128 * (r + 1)]
                        for h in range(2):
                            if split:
                                nc.tensor.matmul(
                                    qb[:, 512 * h : 512 * (h + 1)],
                                    lhsT,
                                    qn[md][64 * g : 64 * (g + 1), 512 * h : 512 * (h + 1)],
                                    start=True,
                                    stop=False,
                                )
                                nc.tensor.matmul(
                                    qb[:, 512 * h : 512 * (h + 1)],
                                    lhsT,
                                    qn_lo[md][64 * g : 64 * (g + 1), 512 * h : 512 * (h + 1)],
                                    start=False,
                                    stop=True,
                                )
                            else:
                                nc.tensor.matmul(
                                    qb[:, 512 * h : 512 * (h + 1)],
                                    mm_dt(lhsT, f32r_sel),
                                    mm_dt(qn[md][64 * g : 64 * (g + 1), 512 * h : 512 * (h + 1)], f32r_sel),
                                    start=True,
                                    stop=True,
                                )
                        t = md * 32 + g * 16 + r
                        if t % 2 == 0:
                            ot = ost.tile([128, 2 * N], F32)
                        nc.vector.tensor_mul(
                            ot[:, (t % 2) * N : (t % 2 + 1) * N], qb[:], xnr[:]
                        )
                        if t % 2 == 1:
                            eng = out_dma_engines[(t // 2) % 2]
                            # DRAM rows 128(t-1)+p (half 0) and 128t+p (half 1)
                            # must match SBUF partition p's two 1024-col halves.
                            dst = out_d[128 * (t - 1) : 128 * (t + 1), :].rearrange(
                                "(h p) n -> p h n", h=2
                            )
                            src = ot[:].rearrange("p (h n) -> p h n", h=2)
                            eng.dma_start(dst, src)

    nc.compile()
    return nc


def _host_inputs(q, x, Wp, bp, g1, b1, g2, b2):
    """Build the 8 per-core input maps."""
    qf = np.ascontiguousarray(np.asarray(q, dtype=np.float32).reshape(B, C1, N))
    xf = np.ascontiguousarray(np.asarray(x, dtype=np.float32).reshape(B, C2, N))
    wpt = np.ascontiguousarray(np.asarray(Wp, dtype=np.float32).T)
    bpc = np.ascontiguousarray(np.asarray(bp, dtype=np.float32).reshape(2, 128).T)
    g1c = np.ascontiguousarray(np.asarray(g1, dtype=np.float32).reshape(2, 128).T)
    b1c = np.ascontiguousarray(np.asarray(b1, dtype=np.float32).reshape(2, 128).T)
    g2r = np.ascontiguousarray(np.tile(np.asarray(g2, dtype=np.float32), 4)[:, None])
    b2r = np.ascontiguousarray(np.tile(np.asarray(b2, dtype=np.float32), 4)[:, None])
    import os

    import ml_dtypes

    # rep[:, r*128+p]: vertical 2x stack of S64_r, S64_r[k,p] = d(k, 4r + p//32)
    rep = np.zeros((128, 16 * 128), dtype=np.float32)
    for r in range(16):
        for p in range(128):
            k = 4 * r + p // 32
            for v in range(2):
                rep[64 * v + k, 128 * r + p] = 1.0
    if os.environ.get("HM_SPLIT", "1") == "1":
        rep = rep.astype(ml_dtypes.bfloat16)
    sx = np.zeros((C2, 128), dtype=np.float32)
    for p in range(128):
        sx[p % 32, p] = 1.0
    in_maps = []
    for b in range(B):
        in_maps.append(
            {
                "q": qf[b],
                "x": xf[b],
                "wpt": wpt,
                "bpc": bpc,
                "g1c": g1c,
                "b1c": b1c,
                "g2r": g2r,
                "b2r": b2r,
                "rep": rep,
                "sx": sx,
            }
        )
    return in_maps


def _run(in_maps, trace=False):
    import os

    from concourse.bass_utils import run_bass_kernel_spmd

    key = "nc" + os.environ.get("HM_SIMPLE", "0")
    if key not in _CACHE:
        _CACHE[key] = _build_nc()
    nc = _CACHE[key]
    res = run_bass_kernel_spmd(
        nc, in_maps, core_ids=list(range(B)), trace=trace
    )
    return res


def kernel(q, x, Wp, bp, g1, b1, g2, b2):
    import os

    simple = (
        np.allclose(np.asarray(bp), 0)
        and np.allclose(np.asarray(g1), 1)
        and np.allclose(np.asarray(b1), 0)
        and np.allclose(np.asarray(g2), 1)
        and np.allclose(np.asarray(b2), 0)
    )
    os.environ["HM_SIMPLE"] = "1" if simple else "0"
    in_maps = _host_inputs(q, x, Wp, bp, g1, b1, g2, b2)
    res = _run(in_maps, trace=False)
    out = np.stack(
        [res.results[b]["out"].reshape(CD, H, W) for b in range(B)]
    ).astype(np.float32)
    _CACHE["last_res"] = res
    return out


# revision 47
# speedup vs baseline: 1.1640x; 1.1640x over previous
"""Trainium2 Bass kernel for nn_HadaMard: fused proj + 2xLayerNorm + outer product.

Reference computation (per batch b):
  qf = q[b].reshape(C1, N)           # [1024, 1024]  (C1 on rows, N=H*W cols)
  proj = Wp @ qf + bp                # [256, 1024]
  qn = LN_over_d(proj) * g1 + b1     # LN over the 256-channel dim
  xn = LN_over_e(x[b]) * g2 + b2     # LN over the 32-channel dim
  out[d*32+e, n] = qn[d, n] * xn[e, n]   # [8192, 1024]

Sharding: data-parallel over B=8, one batch per NeuronCore.

On-chip layout is [channel, n] everywhere (zero transposes):
  - proj = WpT.T @ q via PE (WpT host-transposed, q natural layout)
  - LN stats over the partition axis via ones-matmuls: lhsT = ones*(1/C)
    gives the mean broadcast to all 128 partitions for free.
  - outer product: stationary S4 [4,128] (S4[j,p] = 1 if p//32 == j) broadcasts
    4 qn rows -> 128 partitions in PSUM; one DVE tensor_mul against a
    replicated xn tile -> output tile [128, 1024] -> contiguous 512KB DMA.
"""

import numpy as np

_CACHE = {}

B, C1, H, W = 8, 1024, 32, 32
C2 = 32
Cp = 256
N = H * W  # 1024
CD = Cp * C2  # 8192
EPS = 1e-5


def _build_nc(trace_label=False):
    import os

    import concourse.bacc as bacc
    import concourse.bass as bass
    import concourse.mybir as mybir
    import concourse.tile as tile

    f32r_proj = os.environ.get("HM_F32R_PROJ", "0") == "1"
    f32r_stats = os.environ.get("HM_F32R_STATS", "0") == "1"
    f32r_sel = os.environ.get("HM_F32R_SEL", "0") == "1"
    simple = os.environ.get("HM_SIMPLE", "0") == "1"  # g1=1,b1=0,g2=1,b2=0,bp=0
    split = os.environ.get("HM_SPLIT", "1") == "1"  # bf16 hi/lo selection matmuls

    F32 = mybir.dt.float32
    F32R = mybir.dt.float32r
    BF16 = mybir.dt.bfloat16
    MULT = mybir.AluOpType.mult
    ADD = mybir.AluOpType.add
    SQRT = mybir.ActivationFunctionType.Sqrt

    nc = bacc.Bacc(None, target_bir_lowering=False)

    qh_d = nc.dram_tensor("qh", [C1, N], BF16, kind="ExternalInput")
    ql_d = nc.dram_tensor("ql", [C1, N], BF16, kind="ExternalInput")
    x_d = nc.dram_tensor("x", [C2, N], F32, kind="ExternalInput")
    wh_d = nc.dram_tensor("wh", [C1, Cp], BF16, kind="ExternalInput")
    wl_d = nc.dram_tensor("wl", [C1, Cp], BF16, kind="ExternalInput")
    bp_d = nc.dram_tensor("bpc", [128, 2], F32, kind="ExternalInput")
    g1_d = nc.dram_tensor("g1c", [128, 2], F32, kind="ExternalInput")
    b1_d = nc.dram_tensor("b1c", [128, 2], F32, kind="ExternalInput")
    g2_d = nc.dram_tensor("g2r", [128, 1], F32, kind="ExternalInput")
    b2_d = nc.dram_tensor("b2r", [128, 1], F32, kind="ExternalInput")
    rep_d = nc.dram_tensor(
        "rep", [128, 16 * 128], BF16 if split else F32, kind="ExternalInput"
    )
    sx_d = nc.dram_tensor("sx", [C2, 128], F32, kind="ExternalInput")
    out_d = nc.dram_tensor("out", [CD, N], F32, kind="ExternalOutput")

    with tile.TileContext(nc) as tc:
        with (
            tc.tile_pool(name="cst", bufs=1) as cst,
            tc.tile_pool(name="big", bufs=1) as big,
            tc.tile_pool(name="wrk", bufs=2) as wrk,
            tc.tile_pool(name="stt", bufs=1) as stt,
            tc.tile_pool(name="ost", bufs=4) as ost,
            tc.tile_pool(name="ps", bufs=4, space=bass.MemorySpace.PSUM) as ps,
        ):
            # ---- input loads ----
            qh_sb, ql_sb, wh_sb, wl_sb = [], [], [], []
            for k in range(8):
                t = big.tile([128, N], BF16, tag=f"qh{k}")
                nc.sync.dma_start(t[:], qh_d[128 * k : 128 * (k + 1), :])
                qh_sb.append(t)
                t = big.tile([128, N], BF16, tag=f"ql{k}")
                nc.scalar.dma_start(t[:], ql_d[128 * k : 128 * (k + 1), :])
                ql_sb.append(t)
                t = big.tile([128, Cp], BF16, tag=f"wh{k}")
                nc.sync.dma_start(t[:], wh_d[128 * k : 128 * (k + 1), :])
                wh_sb.append(t)
                t = big.tile([128, Cp], BF16, tag=f"wl{k}")
                nc.scalar.dma_start(t[:], wl_d[128 * k : 128 * (k + 1), :])
                wl_sb.append(t)
            x_sb = cst.tile([C2, N], F32, tag="xs")
            nc.sync.dma_start(x_sb[:], x_d[:])

            def cload(dram, shape, tag):
                t = cst.tile(shape, F32, tag=tag)
                nc.sync.dma_start(t[:], dram[:])
                return t

            bp_sb = cload(bp_d, [128, 2], "bp")
            g1_sb = cload(g1_d, [128, 2], "g1")
            b1_sb = cload(b1_d, [128, 2], "b1")
            g2_sb = cload(g2_d, [128, 1], "g2")
            b2_sb = cload(b2_d, [128, 1], "b2")
            cq_sb = cst.tile([128, 128], F32, tag="cq")
            nc.vector.memset(cq_sb[:], 1.0 / Cp)
            cx_sb = cst.tile([C2, 128], F32, tag="cx")
            nc.vector.memset(cx_sb[:], 1.0 / C2)
            rep_sb = cst.tile([128, 16 * 128], BF16 if split else F32, tag="rep")
            nc.sync.dma_start(rep_sb[:], rep_d[:])
            sx_sb = cload(sx_d, [C2, 128], "sx")
            eps_t = cst.tile([128, 1], F32, tag="eps")
            nc.vector.memset(eps_t[:], EPS)

            def mm_dt(ap, on):
                return ap.bitcast(F32R) if on else ap

            # ---- projection: proj[d, n] = sum_c WpT[c, d] * q[c, n]  (+bp) ----
            projb = []
            for md in range(2):
                pj = ps.tile([128, N], F32, tag="ps")
                for k in range(8):
                    lh = wh_sb[k][:, 128 * md : 128 * (md + 1)]
                    ll = wl_sb[k][:, 128 * md : 128 * (md + 1)]
                    for h in range(2):
                        hs = slice(512 * h, 512 * (h + 1))
                        # wh@qh + wh@ql + wl@qh (ll term ~2^-16, dropped)
                        nc.tensor.matmul(pj[:, hs], lh, qh_sb[k][:, hs],
                                         start=(k == 0), stop=False)
                        nc.tensor.matmul(pj[:, hs], lh, ql_sb[k][:, hs],
                                         start=False, stop=False)
                        nc.tensor.matmul(pj[:, hs], ll, qh_sb[k][:, hs],
                                         start=False, stop=(k == 7))
                pb = stt.tile([128, N], F32, tag=f"pb{md}")
                if simple:
                    nc.vector.tensor_copy(pb[:], pj[:])
                else:
                    nc.vector.tensor_scalar_add(pb[:], pj[:], bp_sb[:, md : md + 1])
                projb.append(pb)

            # squares (ScalarE, keeps DVE free)
            sq = []
            for md in range(2):
                s = wrk.tile([128, N], F32, tag=f"sq{md}")
                nc.scalar.square(s[:], projb[md][:])
                sq.append(s)

            # stats via ones-matmuls: mean & E[v^2], broadcast to 128 partitions
            smq = ps.tile([128, N], F32, tag="ps")
            for md in range(2):
                for h in range(2):
                    nc.tensor.matmul(
                        smq[:, 512 * h : 512 * (h + 1)],
                        mm_dt(cq_sb[:], f32r_stats),
                        mm_dt(projb[md][:, 512 * h : 512 * (h + 1)], f32r_stats),
                        start=(md == 0),
                        stop=(md == 1),
                    )
            sqq = ps.tile([128, N], F32, tag="ps")
            for md in range(2):
                for h in range(2):
                    nc.tensor.matmul(
                        sqq[:, 512 * h : 512 * (h + 1)],
                        mm_dt(cq_sb[:], f32r_stats),
                        mm_dt(sq[md][:, 512 * h : 512 * (h + 1)], f32r_stats),
                        start=(md == 0),
                        stop=(md == 1),
                    )

            mb = stt.tile([128, N], F32, tag="mb")
            nc.vector.tensor_copy(mb[:], smq[:])
            m2 = wrk.tile([128, N], F32, tag="t")
            nc.scalar.square(m2[:], mb[:])
            var = wrk.tile([128, N], F32, tag="t2")
            nc.vector.tensor_sub(var[:], sqq[:], m2[:])
            sd = wrk.tile([128, N], F32, tag="t")
            nc.scalar.activation(sd[:], var[:], SQRT, bias=eps_t[:])
            rstd = stt.tile([128, N], F32, tag="rstd")
            rscr = wrk.tile([128, N], F32, tag="t3")
            nc.vector.reciprocal_approx_accurate(rstd[:], sd[:], rscr[:])

            # simple mode: qn holds (projb - mean); rstd is folded into XR so the
            # per-tile multiply produces (projb-m)*rstd*xn in one op.
            qn = []
            qn_lo = []
            for md in range(2):
                qq = stt.tile([128, N], F32, tag=f"qn{md}")
                nc.vector.tensor_sub(qq[:], projb[md][:], mb[:])
                if not simple:
                    nc.vector.tensor_mul(qq[:], qq[:], rstd[:])
                    nc.vector.tensor_scalar(
                        qq[:], qq[:], g1_sb[:, md : md + 1], b1_sb[:, md : md + 1],
                        op0=MULT, op1=ADD,
                    )
                if split:
                    # bf16 hi/lo decomposition: qq = hi + lo, |lo| <~ 2^-8 |qq|
                    qh = stt.tile([128, N], BF16, tag=f"qh{md}")
                    nc.vector.tensor_copy(qh[:], qq[:])
                    ql = stt.tile([128, N], BF16, tag=f"ql{md}")
                    nc.vector.tensor_sub(ql[:], qq[:], qh[:])
                    qn.append(qh)
                    qn_lo.append(ql)
                else:
                    qn.append(qq)

            # ---- x LayerNorm (over 32 channels) + partition replication ----
            xsq = wrk.tile([C2, N], F32, tag="xq")
            nc.scalar.square(xsq[:], x_sb[:])
            smx = ps.tile([128, N], F32, tag="ps")
            for h in range(2):
                nc.tensor.matmul(
                    smx[:, 512 * h : 512 * (h + 1)], mm_dt(cx_sb[:], f32r_stats),
                    mm_dt(x_sb[:, 512 * h : 512 * (h + 1)], f32r_stats),
                    start=True, stop=True,
                )
            sqx = ps.tile([128, N], F32, tag="ps")
            for h in range(2):
                nc.tensor.matmul(
                    sqx[:, 512 * h : 512 * (h + 1)], mm_dt(cx_sb[:], f32r_stats),
                    mm_dt(xsq[:, 512 * h : 512 * (h + 1)], f32r_stats),
                    start=True, stop=True,
                )
            xb = ps.tile([128, N], F32, tag="ps")
            for h in range(2):
                nc.tensor.matmul(
                    xb[:, 512 * h : 512 * (h + 1)],
                    mm_dt(sx_sb[:], f32r_sel),
                    mm_dt(x_sb[:, 512 * h : 512 * (h + 1)], f32r_sel),
                    start=True, stop=True,
                )

            mxb = wrk.tile([128, N], F32, tag="mx")
            nc.vector.tensor_copy(mxb[:], smx[:])
            mx2 = wrk.tile([128, N], F32, tag="t")
            nc.scalar.square(mx2[:], mxb[:])
            vx = wrk.tile([128, N], F32, tag="t2")
            nc.vector.tensor_sub(vx[:], sqx[:], mx2[:])
            sdx = wrk.tile([128, N], F32, tag="t")
            nc.scalar.activation(sdx[:], vx[:], SQRT, bias=eps_t[:])
            rsx = wrk.tile([128, N], F32, tag="t3")
            rscx = wrk.tile([128, N], F32, tag="t4")
            nc.vector.reciprocal_approx_accurate(rsx[:], sdx[:], rscx[:])
            xt = wrk.tile([128, N], F32, tag="t2")
            nc.vector.tensor_sub(xt[:], xb[:], mxb[:])
            xnr = stt.tile([128, N], F32, tag="xnr")
            nc.vector.tensor_mul(xnr[:], xt[:], rsx[:])
            if simple:
                # fold q-side rstd into the shared multiplier tile
                nc.vector.tensor_mul(xnr[:], xnr[:], rstd[:])
            else:
                nc.vector.tensor_scalar(
                    xnr[:], xnr[:], g2_sb[:, 0:1], b2_sb[:, 0:1], op0=MULT, op1=ADD
                )

            # ---- outer product: 64 output tiles of [128, 1024] ----
            # tile t = (md, g, r): output rows 128t..128(t+1), qn rows
            # 128md + 32g + 4r + {0..3}. lhsT and rhs share base partition 32g
            # (tile_position constraint); rep_sb holds the selection matrices
            # replicated vertically 4x so any 32-row slice works.
            out_dma_engines = [nc.sync, nc.scalar]
            ot = None
            for md in range(2):
                for g in range(2):
                    for r in range(16):
                        qb = ps.tile([128, N], F32, tag="ps")
                        lhsT = rep_sb[64 * g : 64 * (g + 1), 128 * r : 128 * (r + 1)]
                        for h in range(2):
                            if split:
                                nc.tensor.matmul(
                                    qb[:, 512 * h : 512 * (h + 1)],
                                    lhsT,
                                    qn[md][64 * g : 64 * (g + 1), 512 * h : 512 * (h + 1)],
                                    start=True,
                                    stop=False,
                                )
                                nc.tensor.matmul(
                                    qb[:, 512 * h : 512 * (h + 1)],
                                    lhsT,
                                    qn_lo[md][64 * g : 64 * (g + 1), 512 * h : 512 * (h + 1)],
                                    start=False,
                                    stop=True,
                                )
                            else:
                                nc.tensor.matmul(
                                    qb[:, 512 * h : 512 * (h + 1)],
                                    mm_dt(lhsT, f32r_sel),
                                    mm_dt(qn[md][64 * g : 64 * (g + 1), 512 * h : 512 * (h + 1)], f32r_sel),
                                    start=True,
                                    stop=True,
                                )
                        t = md * 32 + g * 16 + r
                        if t % 2 == 0:
                            ot = ost.tile([128, 2 * N], F32)
                        nc.vector.tensor_mul(
                            ot[:, (t % 2) * N : (t % 2 + 1) * N], qb[:], xnr[:]
                        )
                        if t % 2 == 1:
                            eng = out_dma_engines[(t // 2) % 2]
                            # DRAM rows 128(t-1)+p (half 0) and 128t+p (half 1)
                            # must match SBUF partition p's two 1024-col halves.
                            dst = out_d[128 * (t - 1) : 128 * (t + 1), :].rearrange(
                                "(h p) n -> p h n", h=2
                            )
                            src = ot[:].rearrange("p (h n) -> p h n", h=2)
                            eng.dma_start(dst, src)

    nc.compile()
    return nc


def _host_inputs(q, x, Wp, bp, g1, b1, g2, b2):
    """Build the 8 per-core input maps."""
    import os

    import ml_dtypes
    qf = np.ascontiguousarray(np.asarray(q, dtype=np.float32).reshape(B, C1, N))
    qfh = qf.astype(ml_dtypes.bfloat16)
    qfl = (qf - qfh.astype(np.float32)).astype(ml_dtypes.bfloat16)
    xf = np.ascontiguousarray(np.asarray(x, dtype=np.float32).reshape(B, C2, N))
    wpt = np.ascontiguousarray(np.asarray(Wp, dtype=np.float32).T)
    wh = wpt.astype(ml_dtypes.bfloat16)
    wl = (wpt - wh.astype(np.float32)).astype(ml_dtypes.bfloat16)
    bpc = np.ascontiguousarray(np.asarray(bp, dtype=np.float32).reshape(2, 128).T)
    g1c = np.ascontiguousarray(np.asarray(g1, dtype=np.float32).reshape(2, 128).T)
    b1c = np.ascontiguousarray(np.asarray(b1, dtype=np.float32).reshape(2, 128).T)
    g2r = np.ascontiguousarray(np.tile(np.asarray(g2, dtype=np.float32), 4)[:, None])
    b2r = np.ascontiguousarray(np.tile(np.asarray(b2, dtype=np.float32), 4)[:, None])
    # rep[:, r*128+p]: vertical 2x stack of S64_r, S64_r[k,p] = d(k, 4r + p//32)
    rep = np.zeros((128, 16 * 128), dtype=np.float32)
    for r in range(16):
        for p in range(128):
            k = 4 * r + p // 32
            for v in range(2):
                rep[64 * v + k, 128 * r + p] = 1.0
    if os.environ.get("HM_SPLIT", "1") == "1":
        rep = rep.astype(ml_dtypes.bfloat16)
    sx = np.zeros((C2, 128), dtype=np.float32)
    for p in range(128):
        sx[p % 32, p] = 1.0
    in_maps = []
    for b in range(B):
        in_maps.append(
            {
                "qh": np.ascontiguousarray(qfh[b]),
                "ql": np.ascontiguousarray(qfl[b]),
                "x": xf[b],
                "wh": wh,
                "wl": wl,
                "bpc": bpc,
                "g1c": g1c,
                "b1c": b1c,
                "g2r": g2r,
                "b2r": b2r,
                "rep": rep,
                "sx": sx,
            }
        )
    return in_maps


def _run(in_maps, trace=False):
    import os

    from concourse.bass_utils import run_bass_kernel_spmd

    key = "nc" + os.environ.get("HM_SIMPLE", "0")
    if key not in _CACHE:
        _CACHE[key] = _build_nc()
    nc = _CACHE[key]
    res = run_bass_kernel_spmd(
        nc, in_maps, core_ids=list(range(B)), trace=trace
    )
    return res


def kernel(q, x, Wp, bp, g1, b1, g2, b2):
    import os

    simple = (
        np.allclose(np.asarray(bp), 0)
        and np.allclose(np.asarray(g1), 1)
        and np.allclose(np.asarray(b1), 0)
        and np.allclose(np.asarray(g2), 1)
        and np.allclose(np.asarray(b2), 0)
    )
    os.environ["HM_SIMPLE"] = "1" if simple else "0"
    in_maps = _host_inputs(q, x, Wp, bp, g1, b1, g2, b2)
    res = _run(in_maps, trace=False)
    out = np.stack(
        [res.results[b]["out"].reshape(CD, H, W) for b in range(B)]
    ).astype(np.float32)
    _CACHE["last_res"] = res
    return out
